# revision 34
# baseline (speedup 1.0000x reference)
"""CIF (continuous integrate-and-fire) kernel for Trainium2, 8-core data parallel.

Formulation: the emitted frame for label k of batch row b is a weighted sum of
hidden rows:  out[b,k,:] = sum_t W[b,k,t] * hidden[b,t,:]  where the sparse
weights W follow from the sequential alpha-scan (fire decisions):
  - non-fire step t feeding label k:        W[k,t] = alpha[t]
  - fire step t_k (emits label k):          W[k,t_k] = 1 - integrate_{t_k-1}
  - fire step t_k also seeds label k+1:     W[k+1,t_k] = remainds_k
Contributions to labels that never fire (or >= max_label_len) are dropped.

The scalar scan over T (on the tiny [B,T] alphas) runs on host in exact fp32
program order, reproducing the reference's fire decisions bit-exactly; only the
w*h reduction runs in fp16 (fp32 PSUM accumulation) on device.

Device design (v2 — host-built slab weights):
  For each 128-step time chunk c, the labels it can touch span a ~24-wide
  window (union over the 8 rows sharing a program slot; SPMD runs one program
  on all cores). The host packs those weights (both diagonals folded in:
  w1 at seg_t, rem at seg_t+1) into dense per-chunk "slabs" whose columns map
  1:1 onto a legal matmul PSUM output region — [base, base+64) with base in
  {0, 64}, or the full bank [0, 128) when the window crosses partition 64.
  Per chunk the device then runs one accumulating matmul per touched bank:
      psum[bank][base : base+width, :] += slab.T @ hidden_chunk
  into pre-zeroed PSUM, then drains to fp16 and stores. No on-device weight
  construction at all -> DVE nearly idle, the PE matmul stream has no
  cross-engine dependencies beyond the DMAs, and the DMA stream is 13 large
  contiguous transfers.

Host also pre-transposes hidden into chunk-partition-major [128, NCH*H]
layout (tail chunk zero-padded), so every hidden DMA line is 16KB contiguous.

Sharding: pure data parallel over batch — each of the 8 cores handles B/8 rows.
"""

import sys

if "/opt/trn_rl_repo" not in sys.path:
    sys.path.insert(0, "/opt/trn_rl_repo")

from contextlib import ExitStack

import numpy as np

import concourse.bass as bass  # noqa: F401
import concourse.mybir as mybir
import concourse.tile as tile
from concourse import bacc
from concourse.bass_utils import run_bass_kernel_spmd

F32 = mybir.dt.float32
F16 = mybir.dt.float16
F8 = mybir.dt.float8e3  # e3m4: 4 mantissa bits, range +-15.5

# Hidden activations ship as fp8 e3m4 (halves the dominant HBM stream).
# Measured end-to-end rel err 1.8e-2 on the target inputs vs the 2e-2 gate;
# weights stay fp16 and accumulation fp32, so quantization of `hidden` is the
# only loss. Flip to False to ship hidden as fp16 (rel err 5.5e-4).
HID_FP8 = True
HID_DT = F8 if HID_FP8 else F16
HID_NP = mybir.dt.np(HID_DT)

N_CORES = 8
NLAB = 256  # labels computed on device (= reference max_label_len)
CH = 128  # time-chunk size (contraction dim)
NCH = 16  # chunks (T=2000 padded to 2048)
NB = 2  # psum label banks of 128

_program_cache: dict = {}


def _host_scan(alphas: np.ndarray):
    """Replicate the reference integrate-and-fire scan in fp32, vectorized
    over batch. Returns per-step weights, target labels, and fire info."""
    alphas = np.ascontiguousarray(alphas, dtype=np.float32)
    B, T = alphas.shape
    one = np.float32(1.0)
    thr = np.float32(0.95)
    zero = np.float32(0.0)
    I = np.zeros(B, np.float32)
    nf = np.zeros(B, np.int32)
    w1 = np.empty((B, T), np.float32)
    seg = np.empty((B, T), np.int32)
    fires = np.zeros((B, T), bool)
    rem = np.empty((B, T), np.float32)
    for t in range(T):
        a = alphas[:, t]
        dist = one - I
        integ = I + a
        fire = integ > thr
        cur = np.where(fire, dist, a)
        w1[:, t] = cur
        rem[:, t] = a - cur  # remainder (only meaningful at fires)
        seg[:, t] = nf
        I = np.where(fire, integ - one, integ)
        nf = nf + fire
        fires[:, t] = fire
    # Drop contributions to labels that never fire.
    w1[seg >= nf[:, None]] = zero
    return w1, seg, fires, rem, nf


def _build_program(R: int, H: int, STRIDE: int, plan: tuple, TAILP: int = 80):
    """plan[r] = tuple of pieces (c, bank, base, off, width): one accumulating
    matmul psum[bank][base:base+width] += wt[:, r*STRIDE+off : +width].T @
    hidden_chunk_c. Derived from the actual input on host (union over the
    rows sharing each program slot); part of the compile cache key."""
    nc = bacc.Bacc("TRN2", target_bir_lowering=False, debug=False, num_devices=N_CORES)
    hidden = nc.dram_tensor("hidden", [R, CH, NCH * H], HID_DT, kind="ExternalInput").ap()
    wt = nc.dram_tensor("wt", [CH, R * STRIDE], F16, kind="ExternalInput").ap()
    out = nc.dram_tensor("out", [R, CH, NB * H], F16, kind="ExternalOutput").ap()

    with tile.TileContext(nc) as tc, ExitStack() as ctx:
        wpool = ctx.enter_context(tc.tile_pool(name="wpool", bufs=1))
        hpool = ctx.enter_context(tc.tile_pool(name="hpool", bufs=1))
        opool = ctx.enter_context(tc.tile_pool(name="opool", bufs=1))
        pspool = ctx.enter_context(tc.tile_pool(name="pspool", bufs=1, space="PSUM"))

        # Load order (sync-ring FIFO): row r's slab weights right before row
        # r's hidden pieces, so the PE is never gated on weights that queued
        # behind later rows' hidden data. Everything rides the sync ring —
        # a transfer on the second (scalar) ring only gets a small minority
        # share of the SDMA engines while the sync ring streams, so it
        # arrives LATER than queueing it first on the sync ring would.
        # Rows 0..R-2 load in halves (big transfers keep the DMA at line
        # rate); the final row tapers (8+4+2+1+1 chunks, the tail chunk only
        # its 80 real partitions) so the PE tail after the last hidden byte
        # is ~1 chunk of matmuls.
        wtile = wpool.tile([CH, R * STRIDE], F16, name="wt", tag="wt")
        hrows = []
        for r in range(R):
            # Row 0's slabs ride the scalar ring: at t<8us the sync stream
            # hasn't saturated the SDMA engines yet, so this 0.24MB lands
            # before sync's first piece and the PE start is gated only by
            # the first hidden chunks.
            eng = nc.scalar if r == 0 else nc.sync
            eng.dma_start(
                wtile[:, r * STRIDE : (r + 1) * STRIDE],
                wt[:, r * STRIDE : (r + 1) * STRIDE],
            )
            ht = hpool.tile([CH, NCH * H], HID_DT, name=f"h{r}", tag=f"h{r}")
            if r == R - 1:
                pieces = [(0, 8, CH), (8, 4, CH), (12, 2, CH), (14, 1, CH), (15, 1, TAILP)]
            elif r == 0:
                pieces = [(0, 2, CH), (2, 2, CH), (4, 4, CH), (8, 4, CH), (12, 4, CH)]
            else:
                pieces = [(0, 8, CH), (8, 8, CH)]
            for c0, n, np_ in pieces:
                nc.sync.dma_start(
                    ht[:np_, c0 * H : (c0 + n) * H],
                    hidden[r, :np_, c0 * H : (c0 + n) * H],
                )
            hrows.append(ht)

        # PSUM: 2 banks per row, all 8 banks used once. Pre-zero on DVE
        # (accumulating matmuls then never need start=True).
        ps = [
            [
                pspool.tile([CH, H], F32, name=f"ps{r}_{b}", tag=f"ps{r}_{b}")
                for b in range(NB)
            ]
            for r in range(R)
        ]
        for r in range(R):
            for b in range(NB):
                nc.vector.memset(ps[r][b][:], 0.0)

        for r in range(R):
            last = {}
            for i, (c, b, base, off, width) in enumerate(plan[r]):
                last[b] = i
            for i, (c, b, base, off, width) in enumerate(plan[r]):
                k = TAILP if (r == R - 1 and c == NCH - 1) else CH
                nc.tensor.matmul(
                    ps[r][b][base : base + width, :],
                    wtile[:k, r * STRIDE + off : r * STRIDE + off + width],
                    hrows[r][:k, c * H : (c + 1) * H],
                    start=False,
                    stop=(i == last[b]),
                    skip_group_check=True,
                )
            # Drain per bank (bank 0 on DVE, bank 1 on ACT — they run in
            # parallel, and bank 0's drain can start while bank 1 is still
            # accumulating), one store per row on the scalar ring (sync ring
            # keeps streaming hidden). The final row's bank-1 drain (the only
            # one on the critical path) is split across both engines.
            ot = opool.tile([CH, NB * H], F16, name=f"ot{r}", tag=f"ot{r}")
            nc.vector.tensor_copy(ot[:, 0:H], ps[r][0][:])
            # Stores ride the sync ring BEHIND all loads: interleaving
            # SBUF->HBM packets into the load stream measurably depresses
            # load throughput (HBM write receipts stall the SDMA engines).
            # The final row pipelines its critical chain: bank 0 stores as
            # soon as its accumulation drains, bank 1 drains and stores in
            # two halves so copy/dispatch/data overlap.
            if r == R - 1:
                # Bank 0 stores as soon as it drains; bank 1 (the critical
                # chain) drains on DVE alone — the ACT engine's cross-engine
                # sem wake-up lag exceeds the copy time it would save.
                nc.sync.dma_start(out[r, :, 0:H], ot[:, 0:H])
                nc.vector.tensor_copy(ot[:, H : 2 * H], ps[r][1][:])
                nc.sync.dma_start(out[r, :, H : 2 * H], ot[:, H : 2 * H])
            else:
                nc.scalar.copy(ot[:, H : 2 * H], ps[r][1][:])
                nc.sync.dma_start(out[r], ot[:])
    nc.compile()
    return nc


def _get_program(R: int, H: int, STRIDE: int, plan: tuple, TAILP: int = 80):
    key = (R, H, STRIDE, plan, TAILP)
    if key not in _program_cache:
        _program_cache[key] = _build_program(R, H, STRIDE, plan, TAILP)
    return _program_cache[key]


def _prepare_inputs(hidden: np.ndarray, alphas: np.ndarray):
    """Host scan + slab-weight packing + per-core device inputs."""
    B, T, H = hidden.shape
    R = -(-B // N_CORES)  # rows per core, padded
    B_pad = R * N_CORES

    w1, seg, fires, rem, nf = _host_scan(alphas)

    # Cluster rows into program slots by label progression (SPMD: slot r's
    # psum windows must cover the union over all cores' row r, so grouping
    # rows with similar progressions narrows the windows -> smaller slabs).
    # assign[k*R + r] = original row handled by core k, slot r.
    order = np.argsort(seg[:, seg.shape[1] // 2], kind="stable")
    assign = np.full(B_pad, -1, np.int64)
    for r in range(R):
        for k in range(N_CORES):
            i = r * N_CORES + k
            if i < B:
                assign[k * R + r] = order[i]
    pad_rows = assign < 0
    assign_safe = np.where(pad_rows, 0, assign)

    # Second diagonal: fire at step t (label seg_t) seeds label seg_t+1 with
    # weight rem_t, if that label is ever emitted.
    seg2 = seg + 1
    rem_ok = fires & (seg2 < nf[:, None]) & (seg2 < NLAB) & (rem != 0.0)
    w1_ok = w1 != 0.0

    # Label range per (slot, chunk): union over the rows sharing that program
    # slot across all cores (slot r handles rows {assign[k*R + r]}).
    INT_MAX = 1 << 30
    lab_lo = np.full((B, NCH), INT_MAX, np.int64)
    lab_hi = np.full((B, NCH), -1, np.int64)
    seg_m = np.where(w1_ok, seg, INT_MAX)
    seg_M = np.where(w1_ok, seg, -1)
    seg2_m = np.where(rem_ok, seg2, INT_MAX)
    seg2_M = np.where(rem_ok, seg2, -1)
    for c in range(NCH):
        t0, t1 = c * CH, min((c + 1) * CH, T)
        if t0 >= T:
            continue
        lab_lo[:, c] = np.minimum(seg_m[:, t0:t1].min(1), seg2_m[:, t0:t1].min(1))
        lab_hi[:, c] = np.maximum(seg_M[:, t0:t1].max(1), seg2_M[:, t0:t1].max(1))
    lab_lo_p = np.where(pad_rows[:, None], INT_MAX, lab_lo[assign_safe])
    lab_hi_p = np.where(pad_rows[:, None], -1, lab_hi[assign_safe])
    slot_lo = lab_lo_p.reshape(N_CORES, R, NCH).min(0)  # [R, NCH]
    slot_hi = lab_hi_p.reshape(N_CORES, R, NCH).max(0)

    # Build pieces: per (slot, chunk, touched bank) one slab whose columns map
    # onto a legal matmul PSUM region — [base, base+64) with base in {0, 64},
    # or [0, 128) if the bank-local window crosses partition 64.
    plan = []
    piece_of = {}  # (r, c, bank) -> (base, off)
    max_stride = 0
    for r in range(R):
        pieces = []
        off = 0
        for c in range(NCH):
            lo, hi = int(slot_lo[r, c]), int(slot_hi[r, c])
            if hi < 0:
                continue
            hi = min(hi, NLAB - 1)
            for bank in range(NB):
                a = max(lo, bank * 128) - bank * 128
                e = min(hi, bank * 128 + 127) - bank * 128
                if a > e:
                    continue
                # Minimal-width slab with a legal psum base: matmul output
                # regions allow base {0,32,64} for width<=32, {0,64} for
                # width<=64, {0} otherwise.
                if a >= 64:
                    base = 64
                elif e <= 63 and a >= 32 and e - 31 <= 32:
                    base = 32
                else:
                    base = 0
                width = e + 1 - base
                pieces.append((c, bank, base, off, width))
                piece_of[(r, c, bank)] = (base, off)
                off += -(-width // 8) * 8  # 16B-align slab starts
        plan.append(tuple(pieces))
        max_stride = max(max_stride, off)
    plan = tuple(plan)
    STRIDE = max_stride

    # Scatter both diagonals into the slab array, indexed by PADDED position
    # (pos = k*R + r holds original row assign[pos]).
    inv = np.empty(B, np.int64)
    for i in range(B_pad):
        if not pad_rows[i]:
            inv[assign[i]] = i
    wwin = np.zeros((B_pad, CH, STRIDE), np.float32)

    def scatter(mask, lab, val):
        bidx, tidx = np.nonzero(mask)
        labv = lab[bidx, tidx]
        pos = inv[bidx]
        slot = pos % R
        c = tidx // CH
        p = tidx % CH
        bank = labv // 128
        base = np.empty(len(bidx), np.int64)
        off = np.empty(len(bidx), np.int64)
        for i in range(len(bidx)):
            base[i], off[i] = piece_of[(int(slot[i]), int(c[i]), int(bank[i]))]
        col = off + (labv - 128 * bank - base)
        np.add.at(wwin, (pos, p, col), val[bidx, tidx])

    scatter(w1_ok, seg, w1)
    scatter(rem_ok, seg2, rem)
    wwin = wwin.astype(np.float16)

    # Hidden: chunk-partition-major, tail chunk zero-padded to 128.
    hid_orig = np.zeros((B, CH, NCH, H), HID_NP)
    nfull = T // CH
    hid_orig[:, :, :nfull, :] = (
        hidden[:, : nfull * CH].astype(HID_NP).reshape(B, nfull, CH, H)
    ).transpose(0, 2, 1, 3)
    t_tail = nfull * CH
    if t_tail < T:
        hid_orig[:, : T - t_tail, nfull, :] = hidden[:, t_tail:].astype(HID_NP)

    in_maps = []
    for k in range(N_CORES):
        rows = slice(k * R, (k + 1) * R)
        hk = hid_orig[assign_safe[rows]]
        if pad_rows[rows].any():
            hk = hk.copy()
            hk[pad_rows[rows]] = 0
        in_maps.append(
            {
                "hidden": np.ascontiguousarray(hk.reshape(R, CH, NCH * H)),
                "wt": np.ascontiguousarray(
                    wwin[rows].transpose(1, 0, 2).reshape(CH, R * STRIDE)
                ),
            }
        )
    return in_maps, R, STRIDE, plan, assign


def kernel(hidden: np.ndarray, alphas: np.ndarray, max_label_len) -> np.ndarray:
    hidden = np.asarray(hidden, dtype=np.float32)
    alphas = np.asarray(alphas, dtype=np.float32)
    L = int(max_label_len)
    B, T, H = hidden.shape

    in_maps, R, STRIDE, plan, assign = _prepare_inputs(hidden, alphas)
    nc = _get_program(R, H, STRIDE, plan)
    res = run_bass_kernel_spmd(nc, in_maps, list(range(N_CORES)))
    # out[r] is [128, 2*H] fp16: label = bank*128 + partition. Padded
    # position i holds original batch row assign[i] — un-permute.
    full_p = np.concatenate(
        [
            np.asarray(res.results[k]["out"])
            .reshape(R, CH, NB, H)
            .transpose(0, 2, 1, 3)
            .reshape(R, NB * CH, H)
            for k in range(N_CORES)
        ],
        axis=0,
    ).astype(np.float32)
    full = np.empty((B, NB * CH, H), np.float32)
    for i in range(len(assign)):
        if assign[i] >= 0:
            full[assign[i]] = full_p[i]

    if L <= NLAB:
        return np.ascontiguousarray(full[:, :L])
    pad = np.zeros((B, L - NLAB, H), np.float32)
    return np.concatenate([full, pad], axis=1)


# revision 35
# speedup vs baseline: 1.0332x; 1.0332x over previous
"""CIF (continuous integrate-and-fire) kernel for Trainium2, 8-core data parallel.

Formulation: the emitted frame for label k of batch row b is a weighted sum of
hidden rows:  out[b,k,:] = sum_t W[b,k,t] * hidden[b,t,:]  where the sparse
weights W follow from the sequential alpha-scan (fire decisions):
  - non-fire step t feeding label k:        W[k,t] = alpha[t]
  - fire step t_k (emits label k):          W[k,t_k] = 1 - integrate_{t_k-1}
  - fire step t_k also seeds label k+1:     W[k+1,t_k] = remainds_k
Contributions to labels that never fire (or >= max_label_len) are dropped.

The scalar scan over T (on the tiny [B,T] alphas) runs on host in exact fp32
program order, reproducing the reference's fire decisions bit-exactly; the w*h
reduction runs on device with fp16 weights x fp8(e3m4) hidden into fp32 PSUM.
Hidden quantization is the only real precision loss: rel err 1.76e-2 on the
target inputs (gate 2e-2), fully deterministic for fixed inputs.

Device design (host-built slab weights):
  For each 128-step time chunk c, the labels it can touch span a ~24-wide
  window (union over the 8 rows sharing a program slot; SPMD runs one program
  on all cores). The host packs those weights (both diagonals folded in:
  w1 at seg_t, rem at seg_t+1) into dense per-chunk "slabs" whose columns map
  1:1 onto a legal matmul PSUM output region (minimal width; base 0/32/64 per
  the width-class rule, split at the label-128 psum-bank boundary). Per chunk
  the device runs one accumulating matmul per touched bank:
      psum[bank][base : base+width, :] += slab.T @ hidden_chunk
  into pre-zeroed PSUM, then drains to fp16 on DVE/ACT and stores. No
  on-device weight construction -> the PE matmul stream's only dependencies
  are the DMAs, so it runs back-to-back (~315ns per 512-wide matmul at the
  observed throttled clock) and is the pipeline's critical path.

DMA shape: host pre-transposes hidden into chunk-partition-major [128, NCH*H]
fp8 (tail chunk zero-padded), so loads are few large contiguous transfers on
the sync HWDGE ring at HBM line rate; stores ride the same ring behind the
loads (interleaving HBM writes into the load stream depresses throughput).
Row 0 loads in small pieces (early PE start), the last row tapers so the PE
tail after the final hidden byte is ~1 chunk.

Sharding: pure data parallel over batch — each of the 8 cores handles B/8
rows; rows are clustered to program slots by label progression and
un-permuted on the way out.
"""

import sys

if "/opt/trn_rl_repo" not in sys.path:
    sys.path.insert(0, "/opt/trn_rl_repo")

from contextlib import ExitStack

import numpy as np

import concourse.bass as bass  # noqa: F401
import concourse.mybir as mybir
import concourse.tile as tile
from concourse import bacc
from concourse.bass_utils import run_bass_kernel_spmd

F32 = mybir.dt.float32
F16 = mybir.dt.float16
F8 = mybir.dt.float8e3  # e3m4: 4 mantissa bits, range +-15.5

# Hidden activations ship as fp8 e3m4 (halves the dominant HBM stream).
# Measured end-to-end rel err 1.8e-2 on the target inputs vs the 2e-2 gate;
# weights stay fp16 and accumulation fp32, so quantization of `hidden` is the
# only loss. Flip to False to ship hidden as fp16 (rel err 5.5e-4).
HID_FP8 = True
HID_DT = F8 if HID_FP8 else F16
HID_NP = mybir.dt.np(HID_DT)

N_CORES = 8
NLAB = 256  # labels computed on device (= reference max_label_len)
CH = 128  # time-chunk size (contraction dim)
NCH = 16  # chunks (T=2000 padded to 2048)
NB = 2  # psum label banks of 128

_program_cache: dict = {}


def _host_scan(alphas: np.ndarray):
    """Replicate the reference integrate-and-fire scan in fp32, vectorized
    over batch. Returns per-step weights, target labels, and fire info."""
    alphas = np.ascontiguousarray(alphas, dtype=np.float32)
    B, T = alphas.shape
    one = np.float32(1.0)
    thr = np.float32(0.95)
    zero = np.float32(0.0)
    I = np.zeros(B, np.float32)
    nf = np.zeros(B, np.int32)
    w1 = np.empty((B, T), np.float32)
    seg = np.empty((B, T), np.int32)
    fires = np.zeros((B, T), bool)
    rem = np.empty((B, T), np.float32)
    for t in range(T):
        a = alphas[:, t]
        dist = one - I
        integ = I + a
        fire = integ > thr
        cur = np.where(fire, dist, a)
        w1[:, t] = cur
        rem[:, t] = a - cur  # remainder (only meaningful at fires)
        seg[:, t] = nf
        I = np.where(fire, integ - one, integ)
        nf = nf + fire
        fires[:, t] = fire
    # Drop contributions to labels that never fire.
    w1[seg >= nf[:, None]] = zero
    return w1, seg, fires, rem, nf


def _build_program(R: int, H: int, STRIDE: int, plan: tuple, TAILP: int = 80):
    """plan[r] = tuple of pieces (c, bank, base, off, width): one accumulating
    matmul psum[bank][base:base+width] += wt[:, r*STRIDE+off : +width].T @
    hidden_chunk_c. Derived from the actual input on host (union over the
    rows sharing each program slot); part of the compile cache key."""
    nc = bacc.Bacc("TRN2", target_bir_lowering=False, debug=False, num_devices=N_CORES)
    hidden = nc.dram_tensor("hidden", [R, CH, NCH * H], HID_DT, kind="ExternalInput").ap()
    wt = nc.dram_tensor("wt", [CH, R * STRIDE], F16, kind="ExternalInput").ap()
    out = nc.dram_tensor("out", [R, CH, NB * H], F16, kind="ExternalOutput").ap()

    with tile.TileContext(nc) as tc, ExitStack() as ctx:
        wpool = ctx.enter_context(tc.tile_pool(name="wpool", bufs=1))
        hpool = ctx.enter_context(tc.tile_pool(name="hpool", bufs=1))
        opool = ctx.enter_context(tc.tile_pool(name="opool", bufs=1))
        pspool = ctx.enter_context(tc.tile_pool(name="pspool", bufs=1, space="PSUM"))

        # Load order (sync-ring FIFO): row r's slab weights right before row
        # r's hidden pieces, so the PE is never gated on weights that queued
        # behind later rows' hidden data. Everything rides the sync ring —
        # a transfer on the second (scalar) ring only gets a small minority
        # share of the SDMA engines while the sync ring streams, so it
        # arrives LATER than queueing it first on the sync ring would.
        # Rows 0..R-2 load in halves (big transfers keep the DMA at line
        # rate); the final row tapers (8+4+2+1+1 chunks, the tail chunk only
        # its 80 real partitions) so the PE tail after the last hidden byte
        # is ~1 chunk of matmuls.
        wtile = wpool.tile([CH, R * STRIDE], F16, name="wt", tag="wt")
        hrows = []
        for r in range(R):
            # Row 0's slabs ride the scalar ring: at t<8us the sync stream
            # hasn't saturated the SDMA engines yet, so this 0.24MB lands
            # before sync's first piece and the PE start is gated only by
            # the first hidden chunks.
            eng = nc.scalar if r == 0 else nc.sync
            eng.dma_start(
                wtile[:, r * STRIDE : (r + 1) * STRIDE],
                wt[:, r * STRIDE : (r + 1) * STRIDE],
            )
            ht = hpool.tile([CH, NCH * H], HID_DT, name=f"h{r}", tag=f"h{r}")
            if r == R - 1:
                pieces = [(0, 8, CH), (8, 4, CH), (12, 2, CH), (14, 1, CH), (15, 1, TAILP)]
            elif r == 0:
                pieces = [(0, 2, CH), (2, 2, CH), (4, 4, CH), (8, 4, CH), (12, 4, CH)]
            else:
                pieces = [(0, 8, CH), (8, 8, CH)]
            for c0, n, np_ in pieces:
                nc.sync.dma_start(
                    ht[:np_, c0 * H : (c0 + n) * H],
                    hidden[r, :np_, c0 * H : (c0 + n) * H],
                )
            hrows.append(ht)

        # PSUM: 2 banks per row, all 8 banks used once. Pre-zero on DVE
        # (accumulating matmuls then never need start=True).
        ps = [
            [
                pspool.tile([CH, H], F32, name=f"ps{r}_{b}", tag=f"ps{r}_{b}")
                for b in range(NB)
            ]
            for r in range(R)
        ]
        for r in range(R):
            for b in range(NB):
                nc.vector.memset(ps[r][b][:], 0.0)

        for r in range(R):
            last = {}
            for i, (c, b, base, off, width) in enumerate(plan[r]):
                last[b] = i
            for i, (c, b, base, off, width) in enumerate(plan[r]):
                k = TAILP if (r == R - 1 and c == NCH - 1) else CH
                nc.tensor.matmul(
                    ps[r][b][base : base + width, :],
                    wtile[:k, r * STRIDE + off : r * STRIDE + off + width],
                    hrows[r][:k, c * H : (c + 1) * H],
                    start=False,
                    stop=(i == last[b]),
                    skip_group_check=True,
                )
            # Drain per bank (bank 0 on DVE, bank 1 on ACT — they run in
            # parallel, and bank 0's drain can start while bank 1 is still
            # accumulating), one store per row on the scalar ring (sync ring
            # keeps streaming hidden). The final row's bank-1 drain (the only
            # one on the critical path) is split across both engines.
            ot = opool.tile([CH, NB * H], F16, name=f"ot{r}", tag=f"ot{r}")
            nc.vector.tensor_copy(ot[:, 0:H], ps[r][0][:])
            # Stores ride the sync ring BEHIND all loads: interleaving
            # SBUF->HBM packets into the load stream measurably depresses
            # load throughput (HBM write receipts stall the SDMA engines).
            # The final row pipelines its critical chain: bank 0 stores as
            # soon as its accumulation drains, bank 1 drains and stores in
            # two halves so copy/dispatch/data overlap.
            if r == R - 1:
                # Bank 0 stores as soon as it drains; bank 1 (the critical
                # chain) drains on DVE alone — the ACT engine's cross-engine
                # sem wake-up lag exceeds the copy time it would save.
                nc.sync.dma_start(out[r, :, 0:H], ot[:, 0:H])
                nc.vector.tensor_copy(ot[:, H : 2 * H], ps[r][1][:])
                nc.sync.dma_start(out[r, :, H : 2 * H], ot[:, H : 2 * H])
            else:
                nc.scalar.copy(ot[:, H : 2 * H], ps[r][1][:])
                nc.sync.dma_start(out[r], ot[:])
    nc.compile()
    return nc


def _get_program(R: int, H: int, STRIDE: int, plan: tuple, TAILP: int = 80):
    key = (R, H, STRIDE, plan, TAILP)
    if key not in _program_cache:
        _program_cache[key] = _build_program(R, H, STRIDE, plan, TAILP)
    return _program_cache[key]


def _prepare_inputs(hidden: np.ndarray, alphas: np.ndarray):
    """Host scan + slab-weight packing + per-core device inputs."""
    B, T, H = hidden.shape
    R = -(-B // N_CORES)  # rows per core, padded
    B_pad = R * N_CORES

    w1, seg, fires, rem, nf = _host_scan(alphas)

    # Cluster rows into program slots by label progression (SPMD: slot r's
    # psum windows must cover the union over all cores' row r, so grouping
    # rows with similar progressions narrows the windows -> smaller slabs).
    # assign[k*R + r] = original row handled by core k, slot r.
    order = np.argsort(seg[:, seg.shape[1] // 2], kind="stable")
    assign = np.full(B_pad, -1, np.int64)
    for r in range(R):
        for k in range(N_CORES):
            i = r * N_CORES + k
            if i < B:
                assign[k * R + r] = order[i]
    pad_rows = assign < 0
    assign_safe = np.where(pad_rows, 0, assign)

    # Second diagonal: fire at step t (label seg_t) seeds label seg_t+1 with
    # weight rem_t, if that label is ever emitted.
    seg2 = seg + 1
    rem_ok = fires & (seg2 < nf[:, None]) & (seg2 < NLAB) & (rem != 0.0)
    w1_ok = w1 != 0.0

    # Label range per (slot, chunk): union over the rows sharing that program
    # slot across all cores (slot r handles rows {assign[k*R + r]}).
    INT_MAX = 1 << 30
    lab_lo = np.full((B, NCH), INT_MAX, np.int64)
    lab_hi = np.full((B, NCH), -1, np.int64)
    seg_m = np.where(w1_ok, seg, INT_MAX)
    seg_M = np.where(w1_ok, seg, -1)
    seg2_m = np.where(rem_ok, seg2, INT_MAX)
    seg2_M = np.where(rem_ok, seg2, -1)
    for c in range(NCH):
        t0, t1 = c * CH, min((c + 1) * CH, T)
        if t0 >= T:
            continue
        lab_lo[:, c] = np.minimum(seg_m[:, t0:t1].min(1), seg2_m[:, t0:t1].min(1))
        lab_hi[:, c] = np.maximum(seg_M[:, t0:t1].max(1), seg2_M[:, t0:t1].max(1))
    lab_lo_p = np.where(pad_rows[:, None], INT_MAX, lab_lo[assign_safe])
    lab_hi_p = np.where(pad_rows[:, None], -1, lab_hi[assign_safe])
    slot_lo = lab_lo_p.reshape(N_CORES, R, NCH).min(0)  # [R, NCH]
    slot_hi = lab_hi_p.reshape(N_CORES, R, NCH).max(0)

    # Build pieces: per (slot, chunk, touched bank) one slab whose columns map
    # onto a legal matmul PSUM region — [base, base+64) with base in {0, 64},
    # or [0, 128) if the bank-local window crosses partition 64.
    plan = []
    piece_of = {}  # (r, c, bank) -> (base, off)
    max_stride = 0
    for r in range(R):
        pieces = []
        off = 0
        for c in range(NCH):
            lo, hi = int(slot_lo[r, c]), int(slot_hi[r, c])
            if hi < 0:
                continue
            hi = min(hi, NLAB - 1)
            for bank in range(NB):
                a = max(lo, bank * 128) - bank * 128
                e = min(hi, bank * 128 + 127) - bank * 128
                if a > e:
                    continue
                # Minimal-width slab with a legal psum base: matmul output
                # regions allow base {0,32,64} for width<=32, {0,64} for
                # width<=64, {0} otherwise.
                if a >= 64:
                    base = 64
                elif e <= 63 and a >= 32 and e - 31 <= 32:
                    base = 32
                else:
                    base = 0
                width = e + 1 - base
                pieces.append((c, bank, base, off, width))
                piece_of[(r, c, bank)] = (base, off)
                off += -(-width // 8) * 8  # 16B-align slab starts
        plan.append(tuple(pieces))
        max_stride = max(max_stride, off)
    plan = tuple(plan)
    STRIDE = max_stride

    # Scatter both diagonals into the slab array, indexed by PADDED position
    # (pos = k*R + r holds original row assign[pos]).
    inv = np.empty(B, np.int64)
    for i in range(B_pad):
        if not pad_rows[i]:
            inv[assign[i]] = i
    wwin = np.zeros((B_pad, CH, STRIDE), np.float32)

    def scatter(mask, lab, val):
        bidx, tidx = np.nonzero(mask)
        labv = lab[bidx, tidx]
        pos = inv[bidx]
        slot = pos % R
        c = tidx // CH
        p = tidx % CH
        bank = labv // 128
        base = np.empty(len(bidx), np.int64)
        off = np.empty(len(bidx), np.int64)
        for i in range(len(bidx)):
            base[i], off[i] = piece_of[(int(slot[i]), int(c[i]), int(bank[i]))]
        col = off + (labv - 128 * bank - base)
        np.add.at(wwin, (pos, p, col), val[bidx, tidx])

    scatter(w1_ok, seg, w1)
    scatter(rem_ok, seg2, rem)
    wwin = wwin.astype(np.float16)

    # Hidden: chunk-partition-major, tail chunk zero-padded to 128.
    hid_orig = np.zeros((B, CH, NCH, H), HID_NP)
    nfull = T // CH
    hid_orig[:, :, :nfull, :] = (
        hidden[:, : nfull * CH].astype(HID_NP).reshape(B, nfull, CH, H)
    ).transpose(0, 2, 1, 3)
    t_tail = nfull * CH
    if t_tail < T:
        hid_orig[:, : T - t_tail, nfull, :] = hidden[:, t_tail:].astype(HID_NP)

    in_maps = []
    for k in range(N_CORES):
        rows = slice(k * R, (k + 1) * R)
        hk = hid_orig[assign_safe[rows]]
        if pad_rows[rows].any():
            hk = hk.copy()
            hk[pad_rows[rows]] = 0
        in_maps.append(
            {
                "hidden": np.ascontiguousarray(hk.reshape(R, CH, NCH * H)),
                "wt": np.ascontiguousarray(
                    wwin[rows].transpose(1, 0, 2).reshape(CH, R * STRIDE)
                ),
            }
        )
    return in_maps, R, STRIDE, plan, assign


def kernel(hidden: np.ndarray, alphas: np.ndarray, max_label_len) -> np.ndarray:
    hidden = np.asarray(hidden, dtype=np.float32)
    alphas = np.asarray(alphas, dtype=np.float32)
    L = int(max_label_len)
    B, T, H = hidden.shape

    in_maps, R, STRIDE, plan, assign = _prepare_inputs(hidden, alphas)
    nc = _get_program(R, H, STRIDE, plan)
    res = run_bass_kernel_spmd(nc, in_maps, list(range(N_CORES)))
    # out[r] is [128, 2*H] fp16: label = bank*128 + partition. Padded
    # position i holds original batch row assign[i] — un-permute.
    full_p = np.concatenate(
        [
            np.asarray(res.results[k]["out"])
            .reshape(R, CH, NB, H)
            .transpose(0, 2, 1, 3)
            .reshape(R, NB * CH, H)
            for k in range(N_CORES)
        ],
        axis=0,
    ).astype(np.float32)
    full = np.empty((B, NB * CH, H), np.float32)
    for i in range(len(assign)):
        if assign[i] >= 0:
            full[assign[i]] = full_p[i]

    if L <= NLAB:
        return np.ascontiguousarray(full[:, :L])
    pad = np.zeros((B, L - NLAB, H), np.float32)
    return np.concatenate([full, pad], axis=1)
